# revision 7
# baseline (speedup 1.0000x reference)
"""Trainium2 Bass kernel for nn_RecurrentRetention.

Reference computation (per batch row b, T=2048, DIN=D=1024, fp32):
    Q = xq @ Wq ; K = xk @ Wk ; V = xv @ Wv
    ksum[t] = sum_e K[t, e]
    u[t, :] = ksum[t] * V[t, :]   (u[0, :] forced to 0)
    S[t] = GAMMA * S[t-1] + u[t]  (S[-1] = 0)
    out = Q * S

Kernel strategy (8 NeuronCores, data-parallel over batch — one row per core):
  * Algebraic rewrite: ksum = K.sum(-1) = xk @ rowsum(Wk), so the full
    K = xk @ Wk GEMM is never needed (saves 1/3 of the GEMM FLOPs).
  * Everything on-device runs in a transposed [feature, time] layout so the
    contraction dim (DIN) of the two remaining GEMMs sits on SBUF partitions
    and the time recurrence runs along the free dimension.
  * Q^T and V^T GEMMs in bf16 on TensorE with fp32 PSUM accumulation.
  * ksum row is computed by a thin PE GEMV (wks as the 1-column stationary),
    then broadcast across the 128 partitions with a rank-1 ones matmul.
  * The decay recurrence is a native DVE prefix scan (tensor_tensor_scan:
    state = gamma * state + u[t], fp32 state), chunked 4x512 and chained
    through `initial` — no PE involvement, no serial inter-tile chain.
  * Inputs stream in 512-column chunks issued in consumption order so the
    TensorE pipeline starts within a few microseconds.
  * Host side only reshapes/casts/slices: transpose inputs to [DIN, T],
    cast to bf16, fold Wk into its row-sum, transpose the output back.
"""

import numpy as np

GAMMA = 0.9865
B, T, DIN, D = 8, 2048, 1024, 1024
KT = DIN // 128   # contraction tiles
ET = D // 128     # output-feature tiles
NT = T // 512     # time chunks per PSUM bank
N_CORES = 8

_COMPILED_NC = None


def _build_nc():
    import concourse.bacc as bacc
    import concourse.mybir as mybir
    from concourse import tile

    f32 = mybir.dt.float32
    bf16 = mybir.dt.bfloat16
    MULT = mybir.AluOpType.mult
    ADD = mybir.AluOpType.add

    nc = bacc.Bacc("TRN2", target_bir_lowering=False, debug=False,
                   num_devices=N_CORES)

    xqT = nc.dram_tensor("xqT", [DIN, T], bf16, kind="ExternalInput")
    xkT = nc.dram_tensor("xkT", [DIN, T], bf16, kind="ExternalInput")
    xvT = nc.dram_tensor("xvT", [DIN, T], bf16, kind="ExternalInput")
    wq = nc.dram_tensor("wq", [DIN, D], bf16, kind="ExternalInput")
    wv = nc.dram_tensor("wv", [DIN, D], bf16, kind="ExternalInput")
    # wks packed [128, KT]: column k holds rowsum(Wk)[128*k : 128*(k+1)]
    wks = nc.dram_tensor("wks", [128, KT], bf16, kind="ExternalInput")
    ones = nc.dram_tensor("ones", [1, 128], bf16, kind="ExternalInput")
    gam = nc.dram_tensor("gam", [128, T], f32, kind="ExternalInput")
    outT = nc.dram_tensor("outT", [D, T], f32, kind="ExternalOutput")

    def tsl(n):
        return slice(n * 512, (n + 1) * 512)

    with tile.TileContext(nc) as tc:
        with (
            tc.tile_pool(name="resident", bufs=1) as res,
            tc.tile_pool(name="xk_stream", bufs=8) as xkp,
            tc.tile_pool(name="u_pool", bufs=2) as up,
            tc.tile_pool(name="s_pool", bufs=2) as sp,
            tc.tile_pool(name="o_pool", bufs=4) as op,
        ):
            # ---- small constants ------------------------------------------
            wks_t = res.tile([128, KT], bf16, tag="wks", name="wks_t")
            nc.sync.dma_start(wks_t[:], wks[:])
            ones_t = res.tile([1, 128], bf16, tag="ones", name="ones_t")
            nc.sync.dma_start(ones_t[:], ones[:])
            gam_t = res.tile([128, T], f32, tag="gam", name="gam_t")
            nc.sync.dma_start(gam_t[:], gam[:])

            # ---- input chunks, DMA-issued in consumption order ------------
            # wv first (V GEMM stationary), then xv chunk-column 0 so the
            # V GEMM can start immediately; xk interleaves for the ksum GEMV;
            # remaining xv, then the Q-side trails.
            wv_t = [res.tile([128, D], bf16, tag=f"wv{k}", name=f"wv{k}")
                    for k in range(KT)]
            for k in range(KT):
                nc.sync.dma_start(wv_t[k][:], wv[k * 128:(k + 1) * 128, :])

            xv_c = [[res.tile([128, 512], bf16, tag=f"xv{k}_{n}",
                              name=f"xv{k}_{n}") for n in range(NT)]
                    for k in range(KT)]
            xk_c = [[xkp.tile([128, 512], bf16, tag="xk",
                              name=f"xk{k}_{n}") for n in range(NT)]
                    for k in range(KT)]
            for k in range(KT):
                nc.sync.dma_start(xv_c[k][0][:], xvT[k * 128:(k + 1) * 128,
                                                     tsl(0)])
            for n in range(NT):
                for k in range(KT):
                    nc.sync.dma_start(xk_c[k][n][:],
                                      xkT[k * 128:(k + 1) * 128, tsl(n)])
            for n in range(1, NT):
                for k in range(KT):
                    nc.sync.dma_start(xv_c[k][n][:],
                                      xvT[k * 128:(k + 1) * 128, tsl(n)])

            wq_t = [res.tile([128, D], bf16, tag=f"wq{k}", name=f"wq{k}")
                    for k in range(KT)]
            for k in range(KT):
                nc.sync.dma_start(wq_t[k][:], wq[k * 128:(k + 1) * 128, :])
            xq_c = [[res.tile([128, 512], bf16, tag=f"xq{k}_{n}",
                              name=f"xq{k}_{n}") for n in range(NT)]
                    for k in range(KT)]
            for n in range(NT):
                for k in range(KT):
                    nc.sync.dma_start(xq_c[k][n][:],
                                      xqT[k * 128:(k + 1) * 128, tsl(n)])

            ks_row = res.tile([1, T], bf16, tag="ks_row", name="ks_row")
            rep = res.tile([128, T], f32, tag="rep", name="rep")

            # All PSUM pools coexist: 2 + 2 + 2 + 2 banks = 8. No bank-reuse
            # serialization between the ksum phase and the main GEMM stream.
            with (
                tc.tile_pool(name="ps_ks", bufs=2, space="PSUM") as pks,
                tc.tile_pool(name="ps_rep", bufs=2, space="PSUM") as prep,
                tc.tile_pool(name="ps_v", bufs=2, space="PSUM") as pv,
                tc.tile_pool(name="ps_q", bufs=2, space="PSUM") as pq,
            ):
                # ---- ksum GEMV + partition broadcast ----------------------
                for n in range(NT):
                    ks_ps = pks.tile([1, 512], f32, tag="ksps",
                                     name=f"ksps{n}")
                    for k in range(KT):
                        nc.tensor.matmul(ks_ps[:], wks_t[:, k:k + 1],
                                         xk_c[k][n][:],
                                         start=(k == 0), stop=(k == KT - 1))
                    # fp32 PSUM -> bf16 SBUF row (ScalarE keeps DVE free)
                    nc.scalar.copy(ks_row[:, tsl(n)], ks_ps[:])
                for n in range(NT):
                    rep_ps = prep.tile([128, 512], f32, tag="repps",
                                       name=f"repps{n}")
                    nc.tensor.matmul(rep_ps[:], ones_t[:], ks_row[:, tsl(n)],
                                     start=True, stop=True)
                    nc.scalar.copy(rep[:, tsl(n)], rep_ps[:])
                # t=0 never contributes: zero ksum column 0 once, so every
                # e-tile's u inherits the zero.
                nc.gpsimd.memset(rep[:, 0:1], 0.0)

                # ---- main e-tile loop -------------------------------------
                for e in range(ET):
                    esl = slice(e * 128, (e + 1) * 128)
                    u_e = up.tile([128, T], bf16, tag="u", name=f"u{e}")
                    s_e = sp.tile([128, T], f32, tag="s", name=f"s{e}")
                    for n in range(NT):
                        v_ps = pv.tile([128, 512], f32, tag="vps",
                                       name=f"vps{e}_{n}")
                        for k in range(KT):
                            nc.tensor.matmul(v_ps[:], wv_t[k][:, esl],
                                             xv_c[k][n][:],
                                             start=(k == 0),
                                             stop=(k == KT - 1))
                        # u = V^T * ksum  (PSUM x SBUF -> SBUF bf16)
                        nc.vector.tensor_mul(u_e[:, tsl(n)], v_ps[:],
                                             rep[:, tsl(n)])
                        # chained prefix scan chunk:
                        #   state = gamma * state + u[t]
                        nc.vector.tensor_tensor_scan(
                            s_e[:, tsl(n)], gam_t[:, tsl(n)], u_e[:, tsl(n)],
                            0.0 if n == 0 else s_e[:, n * 512 - 1:n * 512],
                            op0=MULT, op1=ADD)
                    for n in range(NT):
                        q_ps = pq.tile([128, 512], f32, tag="qps",
                                       name=f"qps{e}_{n}")
                        for k in range(KT):
                            nc.tensor.matmul(q_ps[:], wq_t[k][:, esl],
                                             xq_c[k][n][:],
                                             start=(k == 0),
                                             stop=(k == KT - 1))
                        o_c = op.tile([128, 512], f32, tag="o",
                                      name=f"o{e}_{n}")
                        nc.vector.tensor_mul(o_c[:], q_ps[:], s_e[:, tsl(n)])
                        nc.sync.dma_start(outT[esl, tsl(n)], o_c[:])

    nc.compile()
    return nc


def _get_nc():
    global _COMPILED_NC
    if _COMPILED_NC is None:
        _COMPILED_NC = _build_nc()
    return _COMPILED_NC


def _make_in_maps(xq, xk, xv, Wq, Wk, Wv):
    import ml_dtypes

    bf16 = ml_dtypes.bfloat16
    wq_b = Wq.astype(bf16)
    wv_b = Wv.astype(bf16)
    # wks packed [128, KT]: column k = rowsum(Wk)[128k : 128k+128]
    wks = np.ascontiguousarray(
        Wk.sum(axis=1, dtype=np.float32).reshape(KT, 128).T).astype(bf16)
    ones = np.ones((1, 128), dtype=bf16)
    gam = np.full((128, T), GAMMA, dtype=np.float32)

    in_maps = []
    for c in range(N_CORES):
        in_maps.append({
            "xqT": np.ascontiguousarray(xq[c].T).astype(bf16),
            "xkT": np.ascontiguousarray(xk[c].T).astype(bf16),
            "xvT": np.ascontiguousarray(xv[c].T).astype(bf16),
            "wq": wq_b,
            "wv": wv_b,
            "wks": wks,
            "ones": ones,
            "gam": gam,
        })
    return in_maps


def run_on_hw(xq, xk, xv, Wq, Wk, Wv, trace=False):
    """Returns (output [B,T,D] fp32, BassKernelResults)."""
    from concourse.bass_utils import run_bass_kernel_spmd

    nc = _get_nc()
    in_maps = _make_in_maps(
        np.asarray(xq), np.asarray(xk), np.asarray(xv),
        np.asarray(Wq), np.asarray(Wk), np.asarray(Wv))
    res = run_bass_kernel_spmd(nc, in_maps, list(range(N_CORES)), trace=trace)
    out = np.empty((B, T, D), dtype=np.float32)
    for c in range(N_CORES):
        out[c] = res.results[c]["outT"].T
    return out, res


def kernel(xq, xk, xv, Wq, Wk, Wv):
    out, _ = run_on_hw(xq, xk, xv, Wq, Wk, Wv, trace=False)
    return out
